# revision 26
# baseline (speedup 1.0000x reference)
"""AttentionPruneViT-Small Trainium2 kernel (Bass/Tile), data-parallel over
batch on 8 NeuronCores (8 images per core).

Self-contained: hardcodes all shapes; host side patchifies the input, folds
LN affines into adjacent weights, lays out weights for the device, runs the
Bass kernel on cores 0-7 and reassembles the [64, 100] output.

Numerics: all matmuls in fp32r (PE rounds operands to 11-bit mantissa RNE,
fp32 accumulate). Softmax without max-subtraction (scores are tiny). Token
pruning implemented by masking pruned keys out of attention (exp bias of
-1e30) -- mathematically identical to the reference's gather given identical
keep sets, which fp32r preserves (validated against fp32 on this input
distribution).
"""
import os
import numpy as np
from contextlib import ExitStack

import concourse.bass as bass
import concourse.mybir as mybir
import concourse.tile as tile
from concourse import bacc
from concourse.bass_utils import run_bass_kernel_spmd

F32 = mybir.dt.float32
F32R = mybir.dt.float32r
AF = mybir.ActivationFunctionType
OP = mybir.AluOpType

# model constants
B = 64
C = 384
HEADS = 6
HD = 64
MLP = 1536
LAYERS = 12
NPATCH = 196
NTOK = 197
NCLS = 100
EPS = 1e-6
PRUNE = {2: 20, 4: 27, 6: 30}   # layer -> number of tokens dropped
SCALE = HD ** -0.5

# per-core geometry
IMGS = 8                 # images per core
W = 208                  # padded token stride per image
CH = [(0, 128), (128, 80)]   # token chunks (offset, width)
NCH = len(CH)
TFLAT = IMGS * W         # 1664
TPAD = TFLAT             # feature-major free size
PAIRS = IMGS // 2
PW = 2 * W               # 416 moving span per image pair
QK_W = PW + 48           # pair Q/K tile free size (S rhs 256-span slack)
NEG = -1e30

N_LAYERS_BUILD = int(os.environ.get("VIT_LAYERS", str(LAYERS)))
PHASE = int(os.environ.get("VIT_PHASE", "9"))
DEBUG_H = os.environ.get("VIT_DEBUG_H", "") == "1"


def _rsqrt(nc, pool, out, var, epsb):
    """out = 1/sqrt(var + eps) on DVE only (magic seed + 2 Newton iters)."""
    P, n = out.shape[0], out.shape[1]
    x = pool.tile([P, n], F32, tag="rsq_x")
    nc.vector.tensor_scalar_add(x[:], var, float(EPS))
    y = pool.tile([P, n], F32, tag="rsq_y")
    xi = x.bitcast(mybir.dt.int32)
    yi = y.bitcast(mybir.dt.int32)
    nc.vector.tensor_scalar(yi[:], xi[:], 1, None, OP.arith_shift_right)
    nc.vector.tensor_scalar(yi[:], yi[:], -1, 0x5f3759df, OP.mult, OP.add)
    t = pool.tile([P, n], F32, tag="rsq_t")
    for it in range(2):
        nc.vector.tensor_tensor(t[:], y[:], y[:], OP.mult)
        nc.vector.tensor_tensor(t[:], t[:], x[:], OP.mult)
        nc.vector.tensor_scalar(t[:], t[:], -0.5, 1.5, OP.mult, OP.add)
        if it == 1:
            nc.vector.tensor_tensor(out[:], y[:], t[:], OP.mult)
        else:
            nc.vector.tensor_tensor(y[:], y[:], t[:], OP.mult)


def build_kernel(n_layers=N_LAYERS_BUILD, debug_h=DEBUG_H):
    nc = bacc.Bacc(target_bir_lowering=False)

    # ---------------- DRAM I/O ----------------
    xp = nc.dram_tensor("xp", [6, 128, IMGS, W], F32R, kind="ExternalInput")
    pw = nc.dram_tensor("pw", [6, 128, C], F32R, kind="ExternalInput")
    h0b = nc.dram_tensor("h0b", [NCH, 128, C], F32, kind="ExternalInput")
    mb0 = nc.dram_tensor("mb0", [NCH, 128], F32, kind="ExternalInput")
    wqk = nc.dram_tensor("wqk", [LAYERS, 3, 128, 768], F32R, kind="ExternalInput")
    bqk = nc.dram_tensor("bqk", [LAYERS, 12, 64], F32, kind="ExternalInput")
    wv = nc.dram_tensor("wv", [LAYERS, 3, 128, 396], F32R, kind="ExternalInput")
    bv = nc.dram_tensor("bv", [LAYERS, 1, 396], F32, kind="ExternalInput")
    wp = nc.dram_tensor("wp", [LAYERS, 6, 65, C], F32R, kind="ExternalInput")
    bp = nc.dram_tensor("bp", [LAYERS, 1, C], F32R, kind="ExternalInput")
    w1 = nc.dram_tensor("w1", [LAYERS, 3, 128, MLP], F32R, kind="ExternalInput")
    b1 = nc.dram_tensor("b1", [LAYERS, 1, MLP], F32, kind="ExternalInput")
    w2 = nc.dram_tensor("w2", [LAYERS, 12, 128, C], F32R, kind="ExternalInput")
    b2 = nc.dram_tensor("b2", [LAYERS, 1, C], F32R, kind="ExternalInput")
    wh = nc.dram_tensor("wh", [3, 128, NCLS], F32R, kind="ExternalInput")
    bh = nc.dram_tensor("bh", [1, NCLS], F32R, kind="ExternalInput")
    out = nc.dram_tensor("out", [IMGS, NCLS], F32, kind="ExternalOutput")
    if debug_h:
        hdbg = nc.dram_tensor("hdbg", [128, IMGS, NCH, C], F32,
                              kind="ExternalOutput")
    # DRAM scratch for cross-partition bounces (prune bookkeeping)
    sc_dram = nc.dram_tensor("sc_dram", [IMGS, NCH * 128], F32)
    dm_dram = nc.dram_tensor("dm_dram", [IMGS, NCH * 128], F32)
    cls_dram = nc.dram_tensor("cls_dram", [IMGS, C], F32)
    den_dram = nc.dram_tensor("den_dram", [IMGS, 1536], F32)
    rden_dram = nc.dram_tensor("rden_dram", [IMGS, 1536], F32)

    with tile.TileContext(nc) as tc, ExitStack() as ctx:
        P = 128
        cpool = ctx.enter_context(tc.tile_pool(name="const", bufs=1))
        spool = ctx.enter_context(tc.tile_pool(name="stats", bufs=2))
        wpool = ctx.enter_context(tc.tile_pool(name="weights", bufs=2))
        wpool1 = ctx.enter_context(tc.tile_pool(name="weights1", bufs=1))
        apool = ctx.enter_context(tc.tile_pool(name="acts", bufs=2))
        qkpool = ctx.enter_context(tc.tile_pool(name="qkp", bufs=1))
        prpool = ctx.enter_context(tc.tile_pool(name="prp", bufs=1))
        vpool = ctx.enter_context(tc.tile_pool(name="vtile", bufs=2))
        ppool = ctx.enter_context(tc.tile_pool(name="ptile", bufs=2))
        opool = ctx.enter_context(tc.tile_pool(name="otile", bufs=2))
        xpool = ctx.enter_context(tc.tile_pool(name="xfm", bufs=1))
        gpool = ctx.enter_context(tc.tile_pool(name="gtile", bufs=1))
        ps1 = ctx.enter_context(tc.tile_pool(name="ps1", bufs=5, space="PSUM"))
        ps3 = ctx.enter_context(tc.tile_pool(name="ps3", bufs=1, space="PSUM"))

        # persistent state
        h = cpool.tile([P, IMGS, NCH, C], F32)            # residual stream
        mb = cpool.tile([P, IMGS, NCH], F32)              # attention key bias
        ones1 = cpool.tile([1, P], F32R)

        from concourse.masks import make_identity
        itmp = prpool.tile([P, P], F32, tag="itmp")
        make_identity(nc, itmp[:])
        identr = cpool.tile([P, P], F32R)
        nc.vector.tensor_copy(identr[:], itmp[:])
        epsb = cpool.tile([P, 1], F32)
        nc.vector.memset(epsb[:], float(EPS))
        nc.vector.memset(h[:], 0.0)
        nc.vector.memset(ones1[:].bitcast(F32), 1.0)
        for b in range(IMGS):
            nc.sync.dma_start(mb[:, b, :], mb0.ap().rearrange("c p -> p c"))

        h0b_t = vpool.tile([P, NCH, C], F32, tag="vt")
        nc.sync.dma_start(h0b_t[:], h0b.ap().rearrange("c p f -> p c f"))

        # ---------------- patch embed ----------------
        pw_t = xpool.tile([P, 6, C], F32R, tag="xfm")
        nc.sync.dma_start(pw_t[:], pw.ap().rearrange("k p f -> p k f"))
        for b in range(IMGS):
            xp_t = gpool.tile([P, 6, W], F32R, tag="g")
            nc.sync.dma_start(xp_t[:], xp.ap()[:, :, b, :].rearrange("k p t -> p k t"))
            for c, (off, wd) in enumerate(CH):
                ps = ps1.tile([P, 512], F32, tag="ps1")
                acc = ps[:wd, :C]
                for kt in range(6):
                    nc.tensor.matmul(acc, xp_t[:, kt, off:off + wd],
                                     pw_t[:, kt, :], start=(kt == 0),
                                     stop=(kt == 5))
                nc.vector.tensor_tensor(h[:wd, b, c, :], acc,
                                        h0b_t[:wd, c, :], OP.add)

        # ---------------- transformer layers ----------------
        for li in range(n_layers):
            # ---- weights for this layer ----
            wqk_t = wpool.tile([P, 3, 768], F32R, tag="wqk")
            nc.sync.dma_start(wqk_t[:], wqk.ap()[li].rearrange("k p m -> p k m"))
            bqk_t = wpool.tile([64, 12], F32, tag="bqk")
            nc.sync.dma_start(bqk_t[:], bqk.ap()[li].rearrange("m p -> p m"))
            wv_t = wpool1.tile([P, 3, 396], F32R, tag="wv")
            nc.sync.dma_start(wv_t[:], wv.ap()[li].rearrange("k p m -> p k m"))
            bv_t = wpool1.tile([1, 396], F32, tag="bv")
            nc.sync.dma_start(bv_t[:], bv.ap()[li])
            bv_m = wpool1.tile([P, 396], F32, tag="bvm")
            nc.gpsimd.partition_broadcast(bv_m[:], bv_t[:])
            wp_t = wpool1.tile([65, 6, C], F32R, tag="wp")
            nc.sync.dma_start(wp_t[:], wp.ap()[li].rearrange("k p m -> p k m"))
            bp_t = wpool1.tile([1, C], F32R, tag="bp")
            nc.sync.dma_start(bp_t[:], bp.ap()[li])
            w1_t = wpool1.tile([P, 3, MLP], F32R, tag="w1")
            for q4 in range(4):
                nc.sync.dma_start(
                    w1_t[:, :, q4 * 384:(q4 + 1) * 384],
                    w1.ap()[li].rearrange("k p m -> p k m")[:, :, q4 * 384:(q4 + 1) * 384])
            b1f_t = wpool1.tile([P, 12], F32, tag="b1")
            nc.sync.dma_start(b1f_t[:], b1.ap()[li].rearrange("o (m p) -> p (o m)", p=P))
            w2_t = wpool1.tile([P, 12, C], F32R, tag="w2")
            for q4 in range(4):
                nc.sync.dma_start(
                    w2_t[:, q4 * 3:(q4 + 1) * 3, :],
                    w2.ap()[li].rearrange("k p m -> p k m")[:, q4 * 3:(q4 + 1) * 3, :])
            b2_t = wpool1.tile([1, C], F32R, tag="b2")
            nc.sync.dma_start(b2_t[:], b2.ap()[li])

            # ---- LN1 + transpose to X_fm ----
            xfm = xpool.tile([P, 3, TPAD], F32R, tag="xfm")

            def layernorm_to_xfm(xfm):
                for pp in range(PAIRS):
                    mv = spool.tile([P, 2, NCH, 2], F32, tag="mv")
                    nc.vector.memset(mv[:], 1.0)
                    for bj in range(2):
                        b = 2 * pp + bj
                        for c, (off, wd) in enumerate(CH):
                            s6 = spool.tile([P, 6], F32, tag="s6")
                            nc.vector.bn_stats(s6[:wd, :], h[:wd, b, c, :])
                            nc.vector.bn_aggr(mv[:wd, bj, c, :], s6[:wd, :])
                    rstd = spool.tile([P, 2 * NCH], F32, tag="rstd")
                    _rsqrt(nc, spool, rstd,
                           mv[:].rearrange("p b c s -> p (b c) s")[:, :, 1],
                           epsb)
                    nmean = spool.tile([P, 2 * NCH], F32, tag="nmean")
                    nc.vector.scalar_tensor_tensor(
                        nmean[:], mv[:].rearrange("p b c s -> p (b c) s")[:, :, 0],
                        -1.0, rstd[:], OP.mult, OP.mult)
                    for bj in range(2):
                        b = 2 * pp + bj
                        for c, (off, wd) in enumerate(CH):
                            xl = apool.tile([P, C], F32R, tag="xln")
                            i = bj * NCH + c
                            nc.scalar.activation(
                                xl[:wd, :], h[:wd, b, c, :], AF.Identity,
                                bias=nmean[:wd, i:i + 1], scale=rstd[:wd, i:i + 1])
                            for f in range(3):
                                pt = ps1.tile([P, 512], F32, tag="ps1")
                                nc.tensor.transpose(
                                    pt[:, :wd].bitcast(F32R),
                                    xl[:wd, f * P:(f + 1) * P],
                                    identr[:wd, :wd])
                                dst = xfm[:, f, b * W + off:b * W + off + wd]
                                if (b * 6 + c * 3 + f) % 2 == 0:
                                    nc.vector.tensor_copy(dst, pt[:, :wd])
                                else:
                                    nc.scalar.copy(dst, pt[:, :wd])

            layernorm_to_xfm(xfm)

            # ---- attention, per image pair ----
            for p in range(PAIRS) if PHASE >= 2 else []:
                b0 = 2 * p
                # Q,K for the pair: [128, 6 mtiles, QK_W]
                qk = qkpool.tile([64, 12, QK_W], F32R, tag="qk")
                nc.vector.memset(qk[:, :, PW:].bitcast(F32), 0.0)
                for m in range(12):
                    pt = ps1.tile([P, 512], F32, tag="ps1")
                    acc = pt[:64, :PW]
                    for kt in range(3):
                        nc.tensor.matmul(acc, wqk_t[:, kt, m * 64:(m + 1) * 64],
                                         xfm[:, kt, b0 * W:b0 * W + PW],
                                         start=(kt == 0), stop=(kt == 2))
                    nc.scalar.activation(qk[:, m, :PW], acc, AF.Identity,
                                         bias=bqk_t[:64, m:m + 1])
                # V for both images: token-major [tok, kc, 6*66]
                vts = []
                for bi in (b0, b0 + 1):
                    vt = vpool.tile([P, NCH, 396], F32R, tag="vt")
                    for c, (off, wd) in enumerate(CH):
                        pv = ps1.tile([P, 512], F32, tag="ps1")
                        acc = pv[:wd, :396]
                        for kt in range(3):
                            nc.tensor.matmul(
                                acc, xfm[:, kt, bi * W + off:bi * W + off + wd],
                                wv_t[:, kt, :], start=(kt == 0), stop=(kt == 2))
                        nc.vector.tensor_tensor(vt[:wd, c, :], acc, bv_m[:wd, :],
                                                OP.add)
                    vts.append(vt)

                for bi in ((b0, b0 + 1) if PHASE >= 3 else []):
                    vt = vts[bi - b0]
                    qoff = (bi - b0) * W
                    # scores S^T and exp
                    pts = []
                    for c, (off, wd) in enumerate(CH):
                        sps = ps3.tile([P, 6, 256], F32, tag="ps3")
                        for hh in range(6):
                            nc.tensor.matmul(
                                sps[:wd, hh, :],
                                qk[:, 6 + hh, qoff + off:qoff + off + wd],
                                qk[:, hh, qoff:qoff + 256],
                                start=True, stop=True)
                        pt = ppool.tile([P, 6, 256], F32R, tag="pt")
                        if PHASE >= 4:
                            nc.scalar.activation(
                                pt[:wd, :, :], sps[:wd, :, :], AF.Exp,
                                bias=mb[:wd, bi, c:c + 1], scale=float(SCALE))
                        else:
                            nc.vector.tensor_copy(pt[:wd, :, :].bitcast(F32), sps[:wd, :, :])
                        pts.append(pt)
                    # AV^T per head-pair: psum [66, 512] holds two heads'
                    # O^T side by side; row 64 = softmax denominator (the
                    # ones column in V). Output is feature-major directly --
                    # no O transposes.
                    if PHASE < 5:
                        continue
                    # row 0 of each head block = softmax denominator (V's
                    # leading ones column); rows 1..64 = O^T values.
                    ofm = opool.tile([65, 6, 256], F32R, tag="ofm")
                    ofmf = ofm[:].rearrange("p h q -> p (h q)")
                    for j in range(3):
                        pav = ps1.tile([P, 512], F32, tag="ps1")
                        for hi in range(2):
                            hh = 2 * j + hi
                            for c, (off, wd) in enumerate(CH):
                                nc.tensor.matmul(
                                    pav[:65, hi * 256:hi * 256 + 256],
                                    vt[:wd, c, hh * 66:hh * 66 + 65],
                                    pts[c][:wd, hh, :],
                                    start=(c == 0), stop=(c == 1))
                        # unnormalized copy to SBUF (row 0 = denominators)
                        nc.vector.tensor_copy(
                            ofmf[:, j * 512:(j + 1) * 512], pav[:65, :])
                        nc.sync.dma_start(
                            den_dram.ap()[bi, j * 512:(j + 1) * 512],
                            ofm[0:1, 2 * j:2 * j + 2, :].bitcast(F32))
                    # denominators -> token-major [q, qc, h]: 128-lane
                    # reciprocal, then back to a feature-major row
                    dtm = prpool.tile([P, 2, 6], F32, tag="dtm")
                    for qc2 in range(2):
                        nc.sync.dma_start(
                            dtm[:, qc2, :],
                            bass.AP(den_dram, bi * 1536 + qc2 * 128,
                                    [[1, 128], [256, 6]]))
                    rtm = prpool.tile([P, 2, 6], F32, tag="rtm")
                    nc.vector.reciprocal(rtm[:], dtm[:])
                    for qc2 in range(2):
                        nc.sync.dma_start(
                            bass.AP(rden_dram, bi * 1536 + qc2 * 128,
                                    [[1, 128], [256, 6]]), rtm[:, qc2, :])
                    for j in range(3):
                        rb = opool.tile([P, 512], F32, tag="rb")
                        nc.sync.dma_start(
                            rb[0:1, :],
                            rden_dram.ap()[bi, j * 512:(j + 1) * 512])
                        nc.gpsimd.partition_broadcast(rb[0:65, :], rb[0:1, :])
                        nc.vector.tensor_tensor(
                            ofmf[:, j * 512:(j + 1) * 512],
                            ofmf[:, j * 512:(j + 1) * 512],
                            rb[0:65, :], OP.mult)
                    # prune scores: weighted CLS column of exp tiles
                    if li in PRUNE:
                        wb = prpool.tile([P, 6], F32, tag="wb")
                        nc.gpsimd.partition_broadcast(wb[:], rtm[0:1, 0, :])
                        sc = prpool.tile([P, NCH], F32, tag="sc")
                        for c, (off, wd) in enumerate(CH):
                            t6 = prpool.tile([P, 6], F32, tag="t6")
                            nc.vector.tensor_tensor(
                                t6[:wd, :], pts[c][:wd, :, 0], wb[:wd, :],
                                OP.mult)
                            nc.vector.reduce_sum(
                                sc[:wd, c:c + 1], t6[:wd, :],
                                axis=mybir.AxisListType.X)
                            nc.sync.dma_start(
                                sc_dram.ap()[bi, c * 128:c * 128 + wd],
                                sc[:wd, c])
                    # proj: 6 K=64 matmuls (one per head) + bias row
                    for qc, (qo, qw) in enumerate(CH):
                        pj = ps1.tile([P, 512], F32, tag="ps1")
                        acc = pj[:qw, :C]
                        for hh in range(6):
                            nc.tensor.matmul(acc, ofm[:, hh, qo:qo + qw],
                                             wp_t[:, hh, :],
                                             start=(hh == 0), stop=False)
                        nc.tensor.matmul(acc, ones1[:, :qw], bp_t[:],
                                         start=False, stop=True)
                        nc.vector.tensor_tensor(h[:qw, bi, qc, :],
                                                h[:qw, bi, qc, :], acc, OP.add)

            # ---- prune mask update ----
            if li in PRUNE:
                drop = PRUNE[li]
                scm = prpool.tile([IMGS, NCH * 128], F32, tag="scm")
                nc.sync.dma_start(scm[:], sc_dram.ap())
                # t = -1e9*(sc==0) - sc  over tokens 1..196
                tneg = prpool.tile([IMGS, NCH * 128], F32, tag="tneg")
                u = prpool.tile([IMGS, NCH * 128], F32, tag="uu")
                nc.vector.tensor_scalar(u[:, 1:NTOK], scm[:, 1:NTOK], 0.0, None,
                                        OP.is_equal)
                nc.vector.scalar_tensor_tensor(
                    tneg[:, 1:NTOK], u[:, 1:NTOK], -1e9, scm[:, 1:NTOK],
                    OP.mult, OP.subtract)
                m8 = prpool.tile([IMGS, 8], F32, tag="m8")
                left = drop
                while left > 0:
                    k = min(8, left)
                    nc.vector.max(m8[:], tneg[:, 1:NTOK])
                    if k < 8:
                        nc.vector.memset(m8[:, k:], -2e30)
                    nc.vector.match_replace(tneg[:, 1:NTOK], m8[:],
                                            tneg[:, 1:NTOK], NEG)
                    left -= k
                dm = prpool.tile([IMGS, NCH * 128], F32, tag="dm")
                nc.vector.memset(dm[:], 0.0)
                nc.vector.tensor_scalar(dm[:, 1:NTOK], tneg[:, 1:NTOK], -1e29,
                                        None, OP.is_le)
                nc.sync.dma_start(dm_dram.ap(), dm[:])
                dmc = prpool.tile([P, IMGS, NCH], F32, tag="dmc")
                for b in range(IMGS):
                    nc.sync.dma_start(
                        dmc[:, b, :],
                        bass.AP(dm_dram, b * NCH * 128, [[1, 128], [128, NCH]]))
                nc.vector.scalar_tensor_tensor(mb[:], dmc[:], NEG, mb[:],
                                               OP.mult, OP.add)

            # ---- LN2 + transpose (reuse xfm) ----
            if PHASE >= 6:
                xfm2 = xpool.tile([P, 3, TPAD], F32R, tag="xfm")
                layernorm_to_xfm(xfm2)

            # ---- MLP per pair ----
            for p in range(PAIRS) if PHASE >= 6 else []:
                b0 = 2 * p
                g = gpool.tile([P, 12, PW], F32R, tag="g")
                for m in range(12):
                    f1 = ps1.tile([P, 512], F32, tag="ps1")
                    acc = f1[:, :PW]
                    for kt in range(3):
                        nc.tensor.matmul(
                            acc, w1_t[:, kt, m * P:(m + 1) * P],
                            xfm2[:, kt, b0 * W:b0 * W + PW],
                            start=(kt == 0), stop=(kt == 2))
                    nc.scalar.activation(g[:, m, :], acc, AF.Gelu,
                                         bias=b1f_t[:, m:m + 1])
                for bi in (b0, b0 + 1):
                    for c, (off, wd) in enumerate(CH):
                        span = (bi - b0) * W + off
                        f2 = ps1.tile([P, 512], F32, tag="ps1")
                        acc = f2[:wd, :C]
                        for kt in range(12):
                            nc.tensor.matmul(acc, g[:, kt, span:span + wd],
                                             w2_t[:, kt, :],
                                             start=(kt == 0), stop=False)
                        nc.tensor.matmul(acc, ones1[:, :wd], b2_t[:],
                                         start=False, stop=True)
                        nc.vector.tensor_tensor(h[:wd, bi, c, :],
                                                h[:wd, bi, c, :], acc, OP.add)

        # ---------------- final LN + head ----------------
        if debug_h:
            nc.sync.dma_start(hdbg.ap(), h[:].rearrange("p b c f -> p b c f"))
        for b in range(IMGS):
            nc.sync.dma_start(cls_dram.ap()[b, :], h[0:1, b, 0, :])
        clst = prpool.tile([IMGS, C], F32, tag="clst")
        nc.sync.dma_start(clst[:], cls_dram.ap())
        s6 = prpool.tile([IMGS, 6], F32, tag="s6f")
        mv = prpool.tile([IMGS, 2], F32, tag="mvf")
        nc.vector.bn_stats(s6[:], clst[:])
        nc.vector.bn_aggr(mv[:], s6[:])
        rstd = prpool.tile([IMGS, 1], F32, tag="rstdf")
        _rsqrt(nc, spool, rstd, mv[:, 1:2], epsb)
        nmean = prpool.tile([IMGS, 1], F32, tag="nmeanf")
        nc.vector.scalar_tensor_tensor(nmean[:], mv[:, 0:1], -1.0, rstd[:],
                                       OP.mult, OP.mult)
        clsn = prpool.tile([IMGS, C], F32R, tag="clsn")
        nc.scalar.activation(clsn[:], clst[:], AF.Identity, bias=nmean[:],
                             scale=rstd[:])
        clsf = prpool.tile([P, 3, IMGS], F32R, tag="clsf")
        for f in range(3):
            pt = ps1.tile([P, 512], F32, tag="ps1")
            nc.tensor.transpose(pt[:, :IMGS].bitcast(F32R),
                                clsn[:, f * P:(f + 1) * P],
                                identr[:IMGS, :IMGS])
            nc.vector.tensor_copy(clsf[:, f, :], pt[:, :IMGS])
        wh_t = prpool.tile([P, 3, NCLS], F32R, tag="wht")
        nc.sync.dma_start(wh_t[:], wh.ap().rearrange("k p m -> p k m"))
        bh_t = prpool.tile([1, NCLS], F32R, tag="bht")
        nc.sync.dma_start(bh_t[:], bh.ap())
        po = ps1.tile([P, 512], F32, tag="ps1")
        acc = po[:IMGS, :NCLS]
        for kt in range(3):
            nc.tensor.matmul(acc, clsf[:, kt, :], wh_t[:, kt, :],
                             start=(kt == 0), stop=False)
        nc.tensor.matmul(acc, ones1[:, :IMGS], bh_t[:], start=False, stop=True)
        ot = prpool.tile([IMGS, NCLS], F32, tag="outf")
        nc.vector.tensor_copy(ot[:], acc)
        nc.sync.dma_start(out.ap(), ot[:])

    nc.finalize()
    return nc


# ======================= host side =======================

def _prep(inputs):
    """Host-side: patchify x, fold LN affines, lay out weights."""
    f32 = np.float32
    d = {}
    x = np.asarray(inputs["x"], f32)
    Bn = x.shape[0]
    # patches feature-major, with token shift (col 0 = CLS placeholder)
    p = x.reshape(Bn, 3, 14, 16, 14, 16).transpose(0, 2, 4, 1, 3, 5)
    p = p.reshape(Bn, NPATCH, 768)
    xp = np.zeros((Bn, 768, W), f32)
    xp[:, :, 1:NTOK] = p.transpose(0, 2, 1)
    d["xp_all"] = xp.reshape(Bn, 6, 128, W)

    pw_ = np.asarray(inputs["patch_w"], f32)
    d["pw"] = pw_.reshape(6, 128, C)

    h0b = np.zeros((NCH, 128, C), f32)
    pos = np.asarray(inputs["pos_embed"], f32)[0]
    pb = np.asarray(inputs["patch_b"], f32)
    cls0 = np.asarray(inputs["cls_token"], f32).reshape(C) + pos[0]
    bias_tok = np.zeros((W, C), f32)
    bias_tok[0] = cls0
    bias_tok[1:NTOK] = pb[None, :] + pos[1:]
    for c, (off, wd) in enumerate(CH):
        h0b[c, :wd] = bias_tok[off:off + wd]
    d["h0b"] = h0b

    mb_ = np.zeros((NCH, 128), f32)
    for c, (off, wd) in enumerate(CH):
        for pp in range(128):
            t = off + pp
            if pp >= wd or t >= NTOK:
                mb_[c, pp] = NEG
    d["mb0"] = mb_

    qkv_w = np.asarray(inputs["qkv_w"], f32)
    qkv_b = np.asarray(inputs["qkv_b"], f32)
    g1 = np.asarray(inputs["ln1_g"], f32)
    b1_ = np.asarray(inputs["ln1_b"], f32)
    g2 = np.asarray(inputs["ln2_g"], f32)
    b2_ = np.asarray(inputs["ln2_b"], f32)

    wqk_l = np.zeros((LAYERS, 3, 128, 768), f32)
    bqk_l = np.zeros((LAYERS, 12, 64), f32)
    wv_l = np.zeros((LAYERS, 3, 128, 396), f32)
    bv_l = np.zeros((LAYERS, 1, 396), f32)
    for li in range(LAYERS):
        wq = qkv_w[li] * g1[li][:, None]          # [C, 3C] folded
        bq = qkv_b[li] + b1_[li] @ qkv_w[li]
        wqk_l[li] = wq[:, :768].reshape(3, 128, 768)
        bqk_l[li] = bq[:768].reshape(12, 64)
        wvl = np.zeros((C, 396), f32)
        bvl = np.zeros((396,), f32)
        for hh in range(HEADS):
            wvl[:, hh * 66 + 1:hh * 66 + 65] = wq[:, 768 + hh * 64:768 + (hh + 1) * 64]
            bvl[hh * 66 + 1:hh * 66 + 65] = bq[768 + hh * 64:768 + (hh + 1) * 64]
            bvl[hh * 66] = 1.0
        wv_l[li] = wvl.reshape(3, 128, 396)
        bv_l[li, 0] = bvl
    d["wqk"], d["bqk"], d["wv"], d["bv"] = wqk_l, bqk_l, wv_l, bv_l

    wp_ = np.zeros((LAYERS, 6, 65, C), f32)
    wp_[:, :, 1:, :] = np.asarray(inputs["proj_w"], f32).reshape(LAYERS, 6, 64, C)
    d["wp"] = wp_
    d["bp"] = np.asarray(inputs["proj_b"], f32).reshape(LAYERS, 1, C)
    w1_ = np.asarray(inputs["fc1_w"], f32) * g2[:, :, None]
    d["w1"] = w1_.reshape(LAYERS, 3, 128, MLP)
    d["b1"] = (np.asarray(inputs["fc1_b"], f32)
               + np.einsum('lc,lcm->lm', b2_, np.asarray(inputs["fc1_w"], f32))
               ).reshape(LAYERS, 1, MLP)
    d["w2"] = np.asarray(inputs["fc2_w"], f32).reshape(LAYERS, 12, 128, C)
    d["b2"] = np.asarray(inputs["fc2_b"], f32).reshape(LAYERS, 1, C)

    ng = np.asarray(inputs["norm_g"], f32)
    nb = np.asarray(inputs["norm_b"], f32)
    hw = np.asarray(inputs["head_w"], f32)
    d["wh"] = (hw * ng[:, None]).reshape(3, 128, NCLS)
    d["bh"] = (np.asarray(inputs["head_b"], f32) + nb @ hw).reshape(1, NCLS)
    return d


_NC_CACHE = {}


def kernel(**inputs):
    key = (N_LAYERS_BUILD, DEBUG_H, PHASE)
    if key not in _NC_CACHE:
        _NC_CACHE[key] = build_kernel()
    nc = _NC_CACHE[key]
    d = _prep(inputs)
    shared = {k: np.ascontiguousarray(v) for k, v in d.items() if k != "xp_all"}
    in_maps = []
    for core in range(8):
        m = dict(shared)
        m["xp"] = np.ascontiguousarray(
            d["xp_all"][core * IMGS:(core + 1) * IMGS].transpose(1, 2, 0, 3))
        in_maps.append(m)
    res = run_bass_kernel_spmd(nc, in_maps, core_ids=list(range(8)))
    outs = [r["out"] for r in res.results]
    return np.concatenate(outs, axis=0)


if __name__ == "__main__":
    rng = np.random.default_rng(0)
    print("building kernel ...")
    nc = build_kernel()
    print("built OK")



# revision 29
# speedup vs baseline: 1.4239x; 1.4239x over previous
"""AttentionPruneViT-Small Trainium2 kernel (Bass/Tile), data-parallel over
batch on 8 NeuronCores (8 images per core).

Self-contained: hardcodes all shapes; host side patchifies the input, folds
LN affines into adjacent weights, lays out weights for the device, runs the
Bass kernel on cores 0-7 and reassembles the [64, 100] output.

Numerics: all matmuls in fp32r (PE rounds operands to 11-bit mantissa RNE,
fp32 accumulate). Softmax without max-subtraction (scores are tiny). Token
pruning implemented by masking pruned keys out of attention (exp bias of
-1e30) -- mathematically identical to the reference's gather given identical
keep sets, which fp32r preserves (validated against fp32 on this input
distribution).
"""
import os
import numpy as np
from contextlib import ExitStack

import concourse.bass as bass
import concourse.mybir as mybir
import concourse.tile as tile
from concourse import bacc
from concourse.bass_utils import run_bass_kernel_spmd

F32 = mybir.dt.float32
F32R = mybir.dt.float32r
AF = mybir.ActivationFunctionType
OP = mybir.AluOpType

# model constants
B = 64
C = 384
HEADS = 6
HD = 64
MLP = 1536
LAYERS = 12
NPATCH = 196
NTOK = 197
NCLS = 100
EPS = 1e-6
PRUNE = {2: 20, 4: 27, 6: 30}   # layer -> number of tokens dropped
SCALE = HD ** -0.5

# per-core geometry
IMGS = 8                 # images per core
W = 208                  # padded token stride per image
CH = [(0, 128), (128, 80)]   # token chunks (offset, width)
NCH = len(CH)
TFLAT = IMGS * W         # 1664
TPAD = TFLAT             # feature-major free size
PAIRS = IMGS // 2
PW = 2 * W               # 416 moving span per image pair
QK_W = PW + 48           # pair Q/K tile free size (S rhs 256-span slack)
NEG = -1e30

N_LAYERS_BUILD = int(os.environ.get("VIT_LAYERS", str(LAYERS)))
PHASE = int(os.environ.get("VIT_PHASE", "9"))
DEBUG_H = os.environ.get("VIT_DEBUG_H", "") == "1"


def _rsqrt(nc, pool, out, var, epsb):
    """out = 1/sqrt(var + eps) on DVE only (magic seed + 2 Newton iters)."""
    P, n = out.shape[0], out.shape[1]
    x = pool.tile([P, n], F32, tag="rsq_x")
    nc.vector.tensor_scalar_add(x[:], var, float(EPS))
    y = pool.tile([P, n], F32, tag="rsq_y")
    xi = x.bitcast(mybir.dt.int32)
    yi = y.bitcast(mybir.dt.int32)
    nc.vector.tensor_scalar(yi[:], xi[:], 1, None, OP.arith_shift_right)
    nc.vector.tensor_scalar(yi[:], yi[:], -1, 0x5f3759df, OP.mult, OP.add)
    t = pool.tile([P, n], F32, tag="rsq_t")
    for it in range(2):
        nc.vector.tensor_tensor(t[:], y[:], y[:], OP.mult)
        nc.vector.tensor_tensor(t[:], t[:], x[:], OP.mult)
        nc.vector.tensor_scalar(t[:], t[:], -0.5, 1.5, OP.mult, OP.add)
        if it == 1:
            nc.vector.tensor_tensor(out[:], y[:], t[:], OP.mult)
        else:
            nc.vector.tensor_tensor(y[:], y[:], t[:], OP.mult)


def build_kernel(n_layers=N_LAYERS_BUILD, debug_h=DEBUG_H):
    nc = bacc.Bacc(target_bir_lowering=False)

    # ---------------- DRAM I/O ----------------
    xp = nc.dram_tensor("xp", [6, 128, IMGS, W], F32R, kind="ExternalInput")
    pw = nc.dram_tensor("pw", [6, 128, C], F32R, kind="ExternalInput")
    h0b = nc.dram_tensor("h0b", [NCH, 128, C], F32, kind="ExternalInput")
    mb0 = nc.dram_tensor("mb0", [NCH, 128], F32, kind="ExternalInput")
    wqk = nc.dram_tensor("wqk", [LAYERS, 3, 128, 768], F32R, kind="ExternalInput")
    bqk = nc.dram_tensor("bqk", [LAYERS, 12, 64], F32, kind="ExternalInput")
    wv = nc.dram_tensor("wv", [LAYERS, 3, 128, 396], F32R, kind="ExternalInput")
    bv = nc.dram_tensor("bv", [LAYERS, 1, 396], F32, kind="ExternalInput")
    wp = nc.dram_tensor("wp", [LAYERS, 6, 65, C], F32R, kind="ExternalInput")
    bp = nc.dram_tensor("bp", [LAYERS, 1, C], F32R, kind="ExternalInput")
    w1 = nc.dram_tensor("w1", [LAYERS, 3, 128, MLP], F32R, kind="ExternalInput")
    b1 = nc.dram_tensor("b1", [LAYERS, 1, MLP], F32, kind="ExternalInput")
    w2 = nc.dram_tensor("w2", [LAYERS, 12, 128, C], F32R, kind="ExternalInput")
    b2 = nc.dram_tensor("b2", [LAYERS, 1, C], F32R, kind="ExternalInput")
    wh = nc.dram_tensor("wh", [3, 128, NCLS], F32R, kind="ExternalInput")
    bh = nc.dram_tensor("bh", [1, NCLS], F32R, kind="ExternalInput")
    out = nc.dram_tensor("out", [IMGS, NCLS], F32, kind="ExternalOutput")
    if debug_h:
        hdbg = nc.dram_tensor("hdbg", [128, IMGS, NCH, C], F32,
                              kind="ExternalOutput")
    # DRAM scratch for cross-partition bounces (prune bookkeeping)
    sc_dram = nc.dram_tensor("sc_dram", [IMGS, NCH * 128], F32)
    dm_dram = nc.dram_tensor("dm_dram", [IMGS, NCH * 128], F32)
    cls_dram = nc.dram_tensor("cls_dram", [IMGS, C], F32)

    with tile.TileContext(nc) as tc, ExitStack() as ctx:
        P = 128
        cpool = ctx.enter_context(tc.tile_pool(name="const", bufs=1))
        spool = ctx.enter_context(tc.tile_pool(name="stats", bufs=2))
        wpool = ctx.enter_context(tc.tile_pool(name="weights", bufs=2))
        wpool1 = ctx.enter_context(tc.tile_pool(name="weights1", bufs=1))
        apool = ctx.enter_context(tc.tile_pool(name="acts", bufs=2))
        qkpool = ctx.enter_context(tc.tile_pool(name="qkp", bufs=1))
        prpool = ctx.enter_context(tc.tile_pool(name="prp", bufs=1))
        vpool = ctx.enter_context(tc.tile_pool(name="vtile", bufs=2))
        ppool = ctx.enter_context(tc.tile_pool(name="ptile", bufs=2))
        opool = ctx.enter_context(tc.tile_pool(name="otile", bufs=2))
        xpool = ctx.enter_context(tc.tile_pool(name="xfm", bufs=1))
        gpool = ctx.enter_context(tc.tile_pool(name="gtile", bufs=1))
        ps1 = ctx.enter_context(tc.tile_pool(name="ps1", bufs=5, space="PSUM"))
        ps3 = ctx.enter_context(tc.tile_pool(name="ps3", bufs=1, space="PSUM"))

        # persistent state
        h = cpool.tile([P, IMGS, NCH, C], F32)            # residual stream
        mb = cpool.tile([P, IMGS, NCH], F32)              # attention key bias
        ones1 = cpool.tile([1, P], F32R)

        from concourse.masks import make_identity
        itmp = prpool.tile([P, P], F32, tag="itmp")
        make_identity(nc, itmp[:])
        identr = cpool.tile([P, P], F32R)
        nc.vector.tensor_copy(identr[:], itmp[:])
        epsb = cpool.tile([P, 1], F32)
        nc.vector.memset(epsb[:], float(EPS))
        nc.vector.memset(h[:], 0.0)
        nc.vector.memset(ones1[:].bitcast(F32), 1.0)
        for b in range(IMGS):
            nc.sync.dma_start(mb[:, b, :], mb0.ap().rearrange("c p -> p c"))

        h0b_t = vpool.tile([P, NCH, C], F32, tag="vt")
        nc.sync.dma_start(h0b_t[:], h0b.ap().rearrange("c p f -> p c f"))

        # ---------------- patch embed ----------------
        pw_t = xpool.tile([P, 6, C], F32R, tag="xfm")
        nc.sync.dma_start(pw_t[:], pw.ap().rearrange("k p f -> p k f"))
        for b in range(IMGS):
            xp_t = gpool.tile([P, 6, W], F32R, tag="g")
            nc.sync.dma_start(xp_t[:], xp.ap()[:, :, b, :].rearrange("k p t -> p k t"))
            for c, (off, wd) in enumerate(CH):
                ps = ps1.tile([P, 512], F32, tag="ps1")
                acc = ps[:wd, :C]
                for kt in range(6):
                    nc.tensor.matmul(acc, xp_t[:, kt, off:off + wd],
                                     pw_t[:, kt, :], start=(kt == 0),
                                     stop=(kt == 5))
                nc.vector.tensor_tensor(h[:wd, b, c, :], acc,
                                        h0b_t[:wd, c, :], OP.add)

        # ---------------- transformer layers ----------------
        for li in range(n_layers):
            # ---- weights for this layer ----
            wqk_t = wpool.tile([P, 3, 768], F32R, tag="wqk")
            nc.sync.dma_start(wqk_t[:], wqk.ap()[li].rearrange("k p m -> p k m"))
            bqk_t = wpool.tile([64, 12], F32, tag="bqk")
            nc.sync.dma_start(bqk_t[:], bqk.ap()[li].rearrange("m p -> p m"))
            wv_t = wpool1.tile([P, 3, 396], F32R, tag="wv")
            nc.sync.dma_start(wv_t[:], wv.ap()[li].rearrange("k p m -> p k m"))
            bv_t = wpool1.tile([1, 396], F32, tag="bv")
            nc.sync.dma_start(bv_t[:], bv.ap()[li])
            bv_m = wpool1.tile([P, 396], F32, tag="bvm")
            nc.gpsimd.partition_broadcast(bv_m[:], bv_t[:])
            wp_t = wpool1.tile([65, 6, C], F32R, tag="wp")
            nc.sync.dma_start(wp_t[:], wp.ap()[li].rearrange("k p m -> p k m"))
            bp_t = wpool1.tile([1, C], F32R, tag="bp")
            nc.sync.dma_start(bp_t[:], bp.ap()[li])
            w1_t = wpool1.tile([P, 3, MLP], F32R, tag="w1")
            for q4 in range(4):
                nc.sync.dma_start(
                    w1_t[:, :, q4 * 384:(q4 + 1) * 384],
                    w1.ap()[li].rearrange("k p m -> p k m")[:, :, q4 * 384:(q4 + 1) * 384])
            b1f_t = wpool1.tile([P, 12], F32, tag="b1")
            nc.sync.dma_start(b1f_t[:], b1.ap()[li].rearrange("o (m p) -> p (o m)", p=P))
            w2_t = wpool1.tile([P, 12, C], F32R, tag="w2")
            for q4 in range(4):
                nc.sync.dma_start(
                    w2_t[:, q4 * 3:(q4 + 1) * 3, :],
                    w2.ap()[li].rearrange("k p m -> p k m")[:, q4 * 3:(q4 + 1) * 3, :])
            b2_t = wpool1.tile([1, C], F32R, tag="b2")
            nc.sync.dma_start(b2_t[:], b2.ap()[li])

            # ---- LN1 + transpose to X_fm ----
            xfm = xpool.tile([P, 3, TPAD], F32R, tag="xfm")

            def layernorm_to_xfm(xfm):
                for pp in range(PAIRS):
                    mv = spool.tile([P, 2, NCH, 2], F32, tag="mv")
                    nc.vector.memset(mv[:], 1.0)
                    for bj in range(2):
                        b = 2 * pp + bj
                        for c, (off, wd) in enumerate(CH):
                            s6 = spool.tile([P, 6], F32, tag="s6")
                            nc.vector.bn_stats(s6[:wd, :], h[:wd, b, c, :])
                            nc.vector.bn_aggr(mv[:wd, bj, c, :], s6[:wd, :])
                    rstd = spool.tile([P, 2 * NCH], F32, tag="rstd")
                    _rsqrt(nc, spool, rstd,
                           mv[:].rearrange("p b c s -> p (b c) s")[:, :, 1],
                           epsb)
                    nmean = spool.tile([P, 2 * NCH], F32, tag="nmean")
                    nc.vector.scalar_tensor_tensor(
                        nmean[:], mv[:].rearrange("p b c s -> p (b c) s")[:, :, 0],
                        -1.0, rstd[:], OP.mult, OP.mult)
                    for bj in range(2):
                        b = 2 * pp + bj
                        for c, (off, wd) in enumerate(CH):
                            xl = apool.tile([P, C], F32R, tag="xln")
                            i = bj * NCH + c
                            nc.scalar.activation(
                                xl[:wd, :], h[:wd, b, c, :], AF.Identity,
                                bias=nmean[:wd, i:i + 1], scale=rstd[:wd, i:i + 1])
                            for f in range(3):
                                pt = ps1.tile([P, 512], F32, tag="ps1")
                                nc.tensor.transpose(
                                    pt[:, :wd].bitcast(F32R),
                                    xl[:wd, f * P:(f + 1) * P],
                                    identr[:wd, :wd])
                                dst = xfm[:, f, b * W + off:b * W + off + wd]
                                if (b * 6 + c * 3 + f) % 2 == 0:
                                    nc.vector.tensor_copy(dst, pt[:, :wd])
                                else:
                                    nc.scalar.copy(dst, pt[:, :wd])

            layernorm_to_xfm(xfm)

            # ---- attention, per image pair ----
            for p in range(PAIRS) if PHASE >= 2 else []:
                b0 = 2 * p
                # Q,K for the pair: [128, 6 mtiles, QK_W]
                qk = qkpool.tile([64, 12, QK_W], F32R, tag="qk")
                nc.vector.memset(qk[:, :, PW:].bitcast(F32), 0.0)
                for m in range(12):
                    pt = ps1.tile([P, 512], F32, tag="ps1")
                    acc = pt[:64, :PW]
                    for kt in range(3):
                        nc.tensor.matmul(acc, wqk_t[:, kt, m * 64:(m + 1) * 64],
                                         xfm[:, kt, b0 * W:b0 * W + PW],
                                         start=(kt == 0), stop=(kt == 2))
                    nc.scalar.activation(qk[:, m, :PW], acc, AF.Identity,
                                         bias=bqk_t[:64, m:m + 1])
                # V for both images: token-major [tok, kc, 6*66]
                vts = []
                for bi in (b0, b0 + 1):
                    vt = vpool.tile([P, NCH, 396], F32R, tag="vt")
                    for c, (off, wd) in enumerate(CH):
                        pv = ps1.tile([P, 512], F32, tag="ps1")
                        acc = pv[:wd, :396]
                        for kt in range(3):
                            nc.tensor.matmul(
                                acc, xfm[:, kt, bi * W + off:bi * W + off + wd],
                                wv_t[:, kt, :], start=(kt == 0), stop=(kt == 2))
                        nc.vector.tensor_tensor(vt[:wd, c, :], acc, bv_m[:wd, :],
                                                OP.add)
                    vts.append(vt)

                for bi in ((b0, b0 + 1) if PHASE >= 3 else []):
                    vt = vts[bi - b0]
                    qoff = (bi - b0) * W
                    # scores S^T and exp
                    pts = []
                    for c, (off, wd) in enumerate(CH):
                        sps = ps3.tile([P, 6, 256], F32, tag="ps3")
                        for hh in range(6):
                            nc.tensor.matmul(
                                sps[:wd, hh, :],
                                qk[:, 6 + hh, qoff + off:qoff + off + wd],
                                qk[:, hh, qoff:qoff + 256],
                                start=True, stop=True)
                        pt = ppool.tile([P, 6, 256], F32R, tag="pt")
                        if PHASE >= 4:
                            nc.scalar.activation(
                                pt[:wd, :, :], sps[:wd, :, :], AF.Exp,
                                bias=mb[:wd, bi, c:c + 1], scale=float(SCALE))
                        else:
                            nc.vector.tensor_copy(pt[:wd, :, :].bitcast(F32), sps[:wd, :, :])
                        pts.append(pt)
                    # AV^T per head-pair: psum [66, 512] holds two heads'
                    # O^T side by side; row 64 = softmax denominator (the
                    # ones column in V). Output is feature-major directly --
                    # no O transposes.
                    if PHASE < 5:
                        continue
                    # row 0 of each head block = softmax denominator (V's
                    # leading ones column); rows 1..64 = O^T values.
                    ofm = opool.tile([65, 6, 256], F32R, tag="ofm")
                    ofmf = ofm[:].rearrange("p h q -> p (h q)")
                    for j in range(3):
                        pav = ps1.tile([P, 512], F32, tag="ps1")
                        for hi in range(2):
                            hh = 2 * j + hi
                            for c, (off, wd) in enumerate(CH):
                                nc.tensor.matmul(
                                    pav[:65, hi * 256:hi * 256 + 256],
                                    vt[:wd, c, hh * 66:hh * 66 + 65],
                                    pts[c][:wd, hh, :],
                                    start=(c == 0), stop=(c == 1))
                        # unnormalized copy to SBUF (row 0 = denominators)
                        nc.vector.tensor_copy(
                            ofmf[:, j * 512:(j + 1) * 512], pav[:65, :])
                    # denominator rows: SBUF->SBUF DMA spreads each row
                    # over 16 lanes, one cheap reciprocal, DMA back to a
                    # lane-0 row for the broadcast multiply.
                    dtm = prpool.tile([16, 3, 32], F32, tag="dtm")
                    for j in range(3):
                        nc.sync.dma_start(
                            dtm[:, j, :],
                            ofm[0:1, 2 * j:2 * j + 2, :].bitcast(F32)
                            .rearrange("p i q -> p (i q)")
                            .rearrange("p (a b) -> p a b", a=16))
                    nc.vector.reciprocal(dtm[:], dtm[:])
                    if li in PRUNE:
                        wbsrc = prpool.tile([1, 6], F32, tag="wbsrc")
                    for j in range(3):
                        rb = opool.tile([P, 512], F32, tag="rb")
                        nc.sync.dma_start(
                            rb[0:1, :].rearrange("p (a b) -> p a b", a=16),
                            dtm[:, j, :])
                        if li in PRUNE:
                            nc.vector.tensor_copy(
                                wbsrc[:, 2 * j:2 * j + 2],
                                rb[0:1, :].rearrange(
                                    "p (i q) -> p i q", i=2)[:, :, 0])
                        nc.gpsimd.partition_broadcast(rb[0:65, :], rb[0:1, :])
                        nc.vector.tensor_tensor(
                            ofmf[:, j * 512:(j + 1) * 512],
                            ofmf[:, j * 512:(j + 1) * 512],
                            rb[0:65, :], OP.mult)
                    # prune scores: weighted CLS column of exp tiles
                    if li in PRUNE:
                        wb = prpool.tile([P, 6], F32, tag="wb")
                        nc.gpsimd.partition_broadcast(wb[:], wbsrc[0:1, :])
                        sc = prpool.tile([P, NCH], F32, tag="sc")
                        for c, (off, wd) in enumerate(CH):
                            t6 = prpool.tile([P, 6], F32, tag="t6")
                            nc.vector.tensor_tensor(
                                t6[:wd, :], pts[c][:wd, :, 0], wb[:wd, :],
                                OP.mult)
                            nc.vector.reduce_sum(
                                sc[:wd, c:c + 1], t6[:wd, :],
                                axis=mybir.AxisListType.X)
                            nc.sync.dma_start(
                                sc_dram.ap()[bi, c * 128:c * 128 + wd],
                                sc[:wd, c])
                    # proj: 6 K=64 matmuls (one per head) + bias row
                    for qc, (qo, qw) in enumerate(CH):
                        pj = ps1.tile([P, 512], F32, tag="ps1")
                        acc = pj[:qw, :C]
                        for hh in range(6):
                            nc.tensor.matmul(acc, ofm[:, hh, qo:qo + qw],
                                             wp_t[:, hh, :],
                                             start=(hh == 0), stop=False)
                        nc.tensor.matmul(acc, ones1[:, :qw], bp_t[:],
                                         start=False, stop=True)
                        nc.vector.tensor_tensor(h[:qw, bi, qc, :],
                                                h[:qw, bi, qc, :], acc, OP.add)

            # ---- prune mask update ----
            if li in PRUNE:
                drop = PRUNE[li]
                scm = prpool.tile([IMGS, NCH * 128], F32, tag="scm")
                nc.sync.dma_start(scm[:], sc_dram.ap())
                # t = -1e9*(sc==0) - sc  over tokens 1..196
                tneg = prpool.tile([IMGS, NCH * 128], F32, tag="tneg")
                u = prpool.tile([IMGS, NCH * 128], F32, tag="uu")
                nc.vector.tensor_scalar(u[:, 1:NTOK], scm[:, 1:NTOK], 0.0, None,
                                        OP.is_equal)
                nc.vector.scalar_tensor_tensor(
                    tneg[:, 1:NTOK], u[:, 1:NTOK], -1e9, scm[:, 1:NTOK],
                    OP.mult, OP.subtract)
                m8 = prpool.tile([IMGS, 8], F32, tag="m8")
                left = drop
                while left > 0:
                    k = min(8, left)
                    nc.vector.max(m8[:], tneg[:, 1:NTOK])
                    if k < 8:
                        nc.vector.memset(m8[:, k:], -2e30)
                    nc.vector.match_replace(tneg[:, 1:NTOK], m8[:],
                                            tneg[:, 1:NTOK], NEG)
                    left -= k
                dm = prpool.tile([IMGS, NCH * 128], F32, tag="dm")
                nc.vector.memset(dm[:], 0.0)
                nc.vector.tensor_scalar(dm[:, 1:NTOK], tneg[:, 1:NTOK], -1e29,
                                        None, OP.is_le)
                nc.sync.dma_start(dm_dram.ap(), dm[:])
                dmc = prpool.tile([P, IMGS, NCH], F32, tag="dmc")
                for b in range(IMGS):
                    nc.sync.dma_start(
                        dmc[:, b, :],
                        bass.AP(dm_dram, b * NCH * 128, [[1, 128], [128, NCH]]))
                nc.vector.scalar_tensor_tensor(mb[:], dmc[:], NEG, mb[:],
                                               OP.mult, OP.add)

            # ---- LN2 + transpose (reuse xfm) ----
            if PHASE >= 6:
                xfm2 = xpool.tile([P, 3, TPAD], F32R, tag="xfm")
                layernorm_to_xfm(xfm2)

            # ---- MLP per pair ----
            for p in range(PAIRS) if PHASE >= 6 else []:
                b0 = 2 * p
                g = gpool.tile([P, 12, PW], F32R, tag="g")
                for m in range(12):
                    f1 = ps1.tile([P, 512], F32, tag="ps1")
                    acc = f1[:, :PW]
                    for kt in range(3):
                        nc.tensor.matmul(
                            acc, w1_t[:, kt, m * P:(m + 1) * P],
                            xfm2[:, kt, b0 * W:b0 * W + PW],
                            start=(kt == 0), stop=(kt == 2))
                    nc.scalar.activation(g[:, m, :], acc, AF.Gelu,
                                         bias=b1f_t[:, m:m + 1])
                for bi in (b0, b0 + 1):
                    for c, (off, wd) in enumerate(CH):
                        span = (bi - b0) * W + off
                        f2 = ps1.tile([P, 512], F32, tag="ps1")
                        acc = f2[:wd, :C]
                        for kt in range(12):
                            nc.tensor.matmul(acc, g[:, kt, span:span + wd],
                                             w2_t[:, kt, :],
                                             start=(kt == 0), stop=False)
                        nc.tensor.matmul(acc, ones1[:, :wd], b2_t[:],
                                         start=False, stop=True)
                        nc.vector.tensor_tensor(h[:wd, bi, c, :],
                                                h[:wd, bi, c, :], acc, OP.add)

        # ---------------- final LN + head ----------------
        if debug_h:
            nc.sync.dma_start(hdbg.ap(), h[:].rearrange("p b c f -> p b c f"))
        for b in range(IMGS):
            nc.sync.dma_start(cls_dram.ap()[b, :], h[0:1, b, 0, :])
        clst = prpool.tile([IMGS, C], F32, tag="clst")
        nc.sync.dma_start(clst[:], cls_dram.ap())
        s6 = prpool.tile([IMGS, 6], F32, tag="s6f")
        mv = prpool.tile([IMGS, 2], F32, tag="mvf")
        nc.vector.bn_stats(s6[:], clst[:])
        nc.vector.bn_aggr(mv[:], s6[:])
        rstd = prpool.tile([IMGS, 1], F32, tag="rstdf")
        _rsqrt(nc, spool, rstd, mv[:, 1:2], epsb)
        nmean = prpool.tile([IMGS, 1], F32, tag="nmeanf")
        nc.vector.scalar_tensor_tensor(nmean[:], mv[:, 0:1], -1.0, rstd[:],
                                       OP.mult, OP.mult)
        clsn = prpool.tile([IMGS, C], F32R, tag="clsn")
        nc.scalar.activation(clsn[:], clst[:], AF.Identity, bias=nmean[:],
                             scale=rstd[:])
        clsf = prpool.tile([P, 3, IMGS], F32R, tag="clsf")
        for f in range(3):
            pt = ps1.tile([P, 512], F32, tag="ps1")
            nc.tensor.transpose(pt[:, :IMGS].bitcast(F32R),
                                clsn[:, f * P:(f + 1) * P],
                                identr[:IMGS, :IMGS])
            nc.vector.tensor_copy(clsf[:, f, :], pt[:, :IMGS])
        wh_t = prpool.tile([P, 3, NCLS], F32R, tag="wht")
        nc.sync.dma_start(wh_t[:], wh.ap().rearrange("k p m -> p k m"))
        bh_t = prpool.tile([1, NCLS], F32R, tag="bht")
        nc.sync.dma_start(bh_t[:], bh.ap())
        po = ps1.tile([P, 512], F32, tag="ps1")
        acc = po[:IMGS, :NCLS]
        for kt in range(3):
            nc.tensor.matmul(acc, clsf[:, kt, :], wh_t[:, kt, :],
                             start=(kt == 0), stop=False)
        nc.tensor.matmul(acc, ones1[:, :IMGS], bh_t[:], start=False, stop=True)
        ot = prpool.tile([IMGS, NCLS], F32, tag="outf")
        nc.vector.tensor_copy(ot[:], acc)
        nc.sync.dma_start(out.ap(), ot[:])

    nc.finalize()
    return nc


# ======================= host side =======================

def _prep(inputs):
    """Host-side: patchify x, fold LN affines, lay out weights."""
    f32 = np.float32
    d = {}
    x = np.asarray(inputs["x"], f32)
    Bn = x.shape[0]
    # patches feature-major, with token shift (col 0 = CLS placeholder)
    p = x.reshape(Bn, 3, 14, 16, 14, 16).transpose(0, 2, 4, 1, 3, 5)
    p = p.reshape(Bn, NPATCH, 768)
    xp = np.zeros((Bn, 768, W), f32)
    xp[:, :, 1:NTOK] = p.transpose(0, 2, 1)
    d["xp_all"] = xp.reshape(Bn, 6, 128, W)

    pw_ = np.asarray(inputs["patch_w"], f32)
    d["pw"] = pw_.reshape(6, 128, C)

    h0b = np.zeros((NCH, 128, C), f32)
    pos = np.asarray(inputs["pos_embed"], f32)[0]
    pb = np.asarray(inputs["patch_b"], f32)
    cls0 = np.asarray(inputs["cls_token"], f32).reshape(C) + pos[0]
    bias_tok = np.zeros((W, C), f32)
    bias_tok[0] = cls0
    bias_tok[1:NTOK] = pb[None, :] + pos[1:]
    for c, (off, wd) in enumerate(CH):
        h0b[c, :wd] = bias_tok[off:off + wd]
    d["h0b"] = h0b

    mb_ = np.zeros((NCH, 128), f32)
    for c, (off, wd) in enumerate(CH):
        for pp in range(128):
            t = off + pp
            if pp >= wd or t >= NTOK:
                mb_[c, pp] = NEG
    d["mb0"] = mb_

    qkv_w = np.asarray(inputs["qkv_w"], f32)
    qkv_b = np.asarray(inputs["qkv_b"], f32)
    g1 = np.asarray(inputs["ln1_g"], f32)
    b1_ = np.asarray(inputs["ln1_b"], f32)
    g2 = np.asarray(inputs["ln2_g"], f32)
    b2_ = np.asarray(inputs["ln2_b"], f32)

    wqk_l = np.zeros((LAYERS, 3, 128, 768), f32)
    bqk_l = np.zeros((LAYERS, 12, 64), f32)
    wv_l = np.zeros((LAYERS, 3, 128, 396), f32)
    bv_l = np.zeros((LAYERS, 1, 396), f32)
    for li in range(LAYERS):
        wq = qkv_w[li] * g1[li][:, None]          # [C, 3C] folded
        bq = qkv_b[li] + b1_[li] @ qkv_w[li]
        wqk_l[li] = wq[:, :768].reshape(3, 128, 768)
        bqk_l[li] = bq[:768].reshape(12, 64)
        wvl = np.zeros((C, 396), f32)
        bvl = np.zeros((396,), f32)
        for hh in range(HEADS):
            wvl[:, hh * 66 + 1:hh * 66 + 65] = wq[:, 768 + hh * 64:768 + (hh + 1) * 64]
            bvl[hh * 66 + 1:hh * 66 + 65] = bq[768 + hh * 64:768 + (hh + 1) * 64]
            bvl[hh * 66] = 1.0
        wv_l[li] = wvl.reshape(3, 128, 396)
        bv_l[li, 0] = bvl
    d["wqk"], d["bqk"], d["wv"], d["bv"] = wqk_l, bqk_l, wv_l, bv_l

    wp_ = np.zeros((LAYERS, 6, 65, C), f32)
    wp_[:, :, 1:, :] = np.asarray(inputs["proj_w"], f32).reshape(LAYERS, 6, 64, C)
    d["wp"] = wp_
    d["bp"] = np.asarray(inputs["proj_b"], f32).reshape(LAYERS, 1, C)
    w1_ = np.asarray(inputs["fc1_w"], f32) * g2[:, :, None]
    d["w1"] = w1_.reshape(LAYERS, 3, 128, MLP)
    d["b1"] = (np.asarray(inputs["fc1_b"], f32)
               + np.einsum('lc,lcm->lm', b2_, np.asarray(inputs["fc1_w"], f32))
               ).reshape(LAYERS, 1, MLP)
    d["w2"] = np.asarray(inputs["fc2_w"], f32).reshape(LAYERS, 12, 128, C)
    d["b2"] = np.asarray(inputs["fc2_b"], f32).reshape(LAYERS, 1, C)

    ng = np.asarray(inputs["norm_g"], f32)
    nb = np.asarray(inputs["norm_b"], f32)
    hw = np.asarray(inputs["head_w"], f32)
    d["wh"] = (hw * ng[:, None]).reshape(3, 128, NCLS)
    d["bh"] = (np.asarray(inputs["head_b"], f32) + nb @ hw).reshape(1, NCLS)
    return d


_NC_CACHE = {}


def kernel(**inputs):
    key = (N_LAYERS_BUILD, DEBUG_H, PHASE)
    if key not in _NC_CACHE:
        _NC_CACHE[key] = build_kernel()
    nc = _NC_CACHE[key]
    d = _prep(inputs)
    shared = {k: np.ascontiguousarray(v) for k, v in d.items() if k != "xp_all"}
    in_maps = []
    for core in range(8):
        m = dict(shared)
        m["xp"] = np.ascontiguousarray(
            d["xp_all"][core * IMGS:(core + 1) * IMGS].transpose(1, 2, 0, 3))
        in_maps.append(m)
    res = run_bass_kernel_spmd(nc, in_maps, core_ids=list(range(8)))
    outs = [r["out"] for r in res.results]
    return np.concatenate(outs, axis=0)


if __name__ == "__main__":
    rng = np.random.default_rng(0)
    print("building kernel ...")
    nc = build_kernel()
    print("built OK")



# revision 31
# speedup vs baseline: 1.5667x; 1.1002x over previous
"""AttentionPruneViT-Small Trainium2 kernel (Bass/Tile), data-parallel over
batch on 8 NeuronCores (8 images per core).

Self-contained: hardcodes all shapes; host side patchifies the input, folds
LN affines into adjacent weights, lays out weights for the device, runs the
Bass kernel on cores 0-7 and reassembles the [64, 100] output.

Numerics: all matmuls in fp32r (PE rounds operands to 11-bit mantissa RNE,
fp32 accumulate). Softmax without max-subtraction (scores are tiny). Token
pruning implemented by masking pruned keys out of attention (exp bias of
-1e30) -- mathematically identical to the reference's gather given identical
keep sets, which fp32r preserves (validated against fp32 on this input
distribution).
"""
import os
import numpy as np
from contextlib import ExitStack

import concourse.bass as bass
import concourse.mybir as mybir
import concourse.tile as tile
from concourse import bacc
from concourse.bass_utils import run_bass_kernel_spmd

F32 = mybir.dt.float32
F32R = mybir.dt.float32r
AF = mybir.ActivationFunctionType
OP = mybir.AluOpType

# model constants
B = 64
C = 384
HEADS = 6
HD = 64
MLP = 1536
LAYERS = 12
NPATCH = 196
NTOK = 197
NCLS = 100
EPS = 1e-6
PRUNE = {2: 20, 4: 27, 6: 30}   # layer -> number of tokens dropped
SCALE = HD ** -0.5

# per-core geometry
IMGS = 8                 # images per core
W = 208                  # padded token stride per image
CH = [(0, 128), (128, 80)]   # token chunks (offset, width)
NCH = len(CH)
TFLAT = IMGS * W         # 1664
TPAD = TFLAT + 64        # feature-major free size (S-matmul 256-span slack)
PAIRS = IMGS // 2
PW = 2 * W               # 416 moving span per image pair
QK_W = PW + 48           # pair Q/K tile free size (S rhs 256-span slack)
NEG = -1e30

N_LAYERS_BUILD = int(os.environ.get("VIT_LAYERS", str(LAYERS)))
PHASE = int(os.environ.get("VIT_PHASE", "9"))
DEBUG_H = os.environ.get("VIT_DEBUG_H", "") == "1"


def _rsqrt_newton(nc, pool, out, var, eps):
    """out = 1/sqrt(var + eps) on DVE only (magic seed + 4 Newton iters).
    var/out: [P, n] f32 SBUF tiles."""
    P, n = var.shape[0], var.shape[1]
    x = pool.tile([P, n], F32, tag="rsq_x")
    nc.vector.tensor_scalar_add(x[:], var[:], float(eps))
    y = pool.tile([P, n], F32, tag="rsq_y")
    # seed: y = magic - (x >> 1) on int32 view
    xi = x.bitcast(mybir.dt.int32)
    yi = y.bitcast(mybir.dt.int32)
    nc.vector.tensor_scalar(yi[:], xi[:], 1, None, OP.arith_shift_right)
    nc.vector.tensor_scalar(yi[:], yi[:], -1, 0x5f3759df,
                            OP.mult, OP.add)
    t = pool.tile([P, n], F32, tag="rsq_t")
    for _ in range(2):
        # t = x * y * y ; y = y * (1.5 - 0.5 * t)
        nc.vector.tensor_tensor(t[:], y[:], y[:], OP.mult)
        nc.vector.tensor_tensor(t[:], t[:], x[:], OP.mult)
        nc.vector.tensor_scalar(t[:], t[:], -0.5, 1.5, OP.mult, OP.add)
        nc.vector.tensor_tensor(y[:], y[:], t[:], OP.mult)
    nc.vector.tensor_copy(out[:], y[:])


def build_kernel(n_layers=N_LAYERS_BUILD, debug_h=DEBUG_H):
    nc = bacc.Bacc(target_bir_lowering=False)

    # ---------------- DRAM I/O ----------------
    xp = nc.dram_tensor("xp", [6, 128, IMGS, W], F32R, kind="ExternalInput")
    pw = nc.dram_tensor("pw", [6, 128, C], F32R, kind="ExternalInput")
    h0b = nc.dram_tensor("h0b", [NCH, 128, C], F32, kind="ExternalInput")
    mb0 = nc.dram_tensor("mb0", [NCH, 128], F32, kind="ExternalInput")
    wqk = nc.dram_tensor("wqk", [LAYERS, 3, 128, 768], F32R, kind="ExternalInput")
    bqk = nc.dram_tensor("bqk", [LAYERS, 12, 64], F32, kind="ExternalInput")
    wv = nc.dram_tensor("wv", [LAYERS, 3, 128, 396], F32R, kind="ExternalInput")
    bv = nc.dram_tensor("bv", [LAYERS, 1, 396], F32, kind="ExternalInput")
    wp = nc.dram_tensor("wp", [LAYERS, 3, 128, C], F32R, kind="ExternalInput")
    bp = nc.dram_tensor("bp", [LAYERS, 1, C], F32R, kind="ExternalInput")
    w1 = nc.dram_tensor("w1", [LAYERS, 3, 128, MLP], F32R, kind="ExternalInput")
    b1 = nc.dram_tensor("b1", [LAYERS, 1, MLP], F32, kind="ExternalInput")
    w2 = nc.dram_tensor("w2", [LAYERS, 12, 128, C], F32R, kind="ExternalInput")
    b2 = nc.dram_tensor("b2", [LAYERS, 1, C], F32R, kind="ExternalInput")
    wh = nc.dram_tensor("wh", [3, 128, NCLS], F32R, kind="ExternalInput")
    bh = nc.dram_tensor("bh", [1, NCLS], F32R, kind="ExternalInput")
    out = nc.dram_tensor("out", [IMGS, NCLS], F32, kind="ExternalOutput")
    if debug_h:
        hdbg = nc.dram_tensor("hdbg", [128, IMGS, NCH, C], F32,
                              kind="ExternalOutput")
    # DRAM scratch for cross-partition bounces (prune bookkeeping)
    sc_dram = nc.dram_tensor("sc_dram", [IMGS, NCH * 128], F32)
    dm_dram = nc.dram_tensor("dm_dram", [IMGS, NCH * 128], F32)
    cls_dram = nc.dram_tensor("cls_dram", [IMGS, C], F32)

    with tile.TileContext(nc) as tc, ExitStack() as ctx:
        P = 128
        cpool = ctx.enter_context(tc.tile_pool(name="const", bufs=1))
        spool = ctx.enter_context(tc.tile_pool(name="stats", bufs=2))
        wpool = ctx.enter_context(tc.tile_pool(name="weights", bufs=2))
        wpool1 = ctx.enter_context(tc.tile_pool(name="weights1", bufs=1))
        apool = ctx.enter_context(tc.tile_pool(name="acts", bufs=2))
        qkpool = ctx.enter_context(tc.tile_pool(name="qkp", bufs=1))
        prpool = ctx.enter_context(tc.tile_pool(name="prp", bufs=1))
        vpool = ctx.enter_context(tc.tile_pool(name="vtile", bufs=2))
        ppool = ctx.enter_context(tc.tile_pool(name="ptile", bufs=2))
        opool = ctx.enter_context(tc.tile_pool(name="otile", bufs=2))
        xpool = ctx.enter_context(tc.tile_pool(name="xfm", bufs=1))
        gpool = ctx.enter_context(tc.tile_pool(name="gtile", bufs=1))
        ps1 = ctx.enter_context(tc.tile_pool(name="ps1", bufs=5, space="PSUM"))
        ps3 = ctx.enter_context(tc.tile_pool(name="ps3", bufs=1, space="PSUM"))

        # persistent state
        h = cpool.tile([P, IMGS, NCH, C], F32)            # residual stream
        mb = cpool.tile([P, IMGS, NCH], F32)              # attention key bias
        ident = cpool.tile([P, P], F32)
        ones1 = cpool.tile([1, P], F32R)

        from concourse.masks import make_identity
        make_identity(nc, ident[:])
        identr = cpool.tile([P, P], F32R)
        nc.vector.tensor_copy(identr[:], ident[:])
        nc.vector.memset(h[:], 0.0)
        nc.vector.memset(ones1[:].bitcast(F32), 1.0)
        for b in range(IMGS):
            nc.sync.dma_start(mb[:, b, :], mb0.ap().rearrange("c p -> p c"))

        h0b_t = vpool.tile([P, NCH, C], F32, tag="vt")
        nc.sync.dma_start(h0b_t[:], h0b.ap().rearrange("c p f -> p c f"))

        # ---------------- patch embed ----------------
        pw_t = xpool.tile([P, 6, C], F32R, tag="xfm")
        nc.sync.dma_start(pw_t[:], pw.ap().rearrange("k p f -> p k f"))
        for b in range(IMGS):
            xp_t = gpool.tile([P, 6, W], F32R, tag="g")
            nc.sync.dma_start(xp_t[:], xp.ap()[:, :, b, :].rearrange("k p t -> p k t"))
            for c, (off, wd) in enumerate(CH):
                ps = ps1.tile([P, 512], F32, tag="ps1")
                acc = ps[:wd, :C]
                for kt in range(6):
                    nc.tensor.matmul(acc, xp_t[:, kt, off:off + wd],
                                     pw_t[:, kt, :], start=(kt == 0),
                                     stop=(kt == 5))
                nc.vector.tensor_tensor(h[:wd, b, c, :], acc,
                                        h0b_t[:wd, c, :], OP.add)

        # ---------------- transformer layers ----------------
        for li in range(n_layers):
            # ---- weights for this layer ----
            wqk_t = wpool.tile([P, 3, 768], F32R, tag="wqk")
            nc.sync.dma_start(wqk_t[:], wqk.ap()[li].rearrange("k p m -> p k m"))
            bqk_t = wpool.tile([64, 12], F32, tag="bqk")
            nc.sync.dma_start(bqk_t[:], bqk.ap()[li].rearrange("m p -> p m"))
            wv_t = wpool1.tile([P, 3, 396], F32R, tag="wv")
            nc.sync.dma_start(wv_t[:], wv.ap()[li].rearrange("k p m -> p k m"))
            bv_t = wpool1.tile([1, 396], F32, tag="bv")
            nc.sync.dma_start(bv_t[:], bv.ap()[li])
            bv_m = wpool1.tile([P, 396], F32, tag="bvm")
            nc.gpsimd.partition_broadcast(bv_m[:], bv_t[:])
            wp_t = wpool1.tile([P, 3, C], F32R, tag="wp")
            nc.sync.dma_start(wp_t[:], wp.ap()[li].rearrange("k p m -> p k m"))
            bp_t = wpool1.tile([1, C], F32R, tag="bp")
            nc.sync.dma_start(bp_t[:], bp.ap()[li])
            w1_t = wpool1.tile([P, 3, MLP], F32R, tag="w1")
            for q4 in range(4):
                nc.sync.dma_start(
                    w1_t[:, :, q4 * 384:(q4 + 1) * 384],
                    w1.ap()[li].rearrange("k p m -> p k m")[:, :, q4 * 384:(q4 + 1) * 384])
            b1f_t = wpool1.tile([P, 12], F32, tag="b1")
            nc.sync.dma_start(b1f_t[:], b1.ap()[li].rearrange("o (m p) -> p (o m)", p=P))
            w2_t = wpool1.tile([P, 12, C], F32R, tag="w2")
            for q4 in range(4):
                nc.sync.dma_start(
                    w2_t[:, q4 * 3:(q4 + 1) * 3, :],
                    w2.ap()[li].rearrange("k p m -> p k m")[:, q4 * 3:(q4 + 1) * 3, :])
            b2_t = wpool1.tile([1, C], F32R, tag="b2")
            nc.sync.dma_start(b2_t[:], b2.ap()[li])

            # ---- LN1 + transpose to X_fm ----
            xfm = xpool.tile([P, 3, TPAD], F32R, tag="xfm")

            def layernorm_to_xfm(xfm):
                for pp in range(PAIRS):
                    mv = spool.tile([P, 2, NCH, 2], F32, tag="mv")
                    nc.vector.memset(mv[:], 1.0)
                    for bj in range(2):
                        b = 2 * pp + bj
                        for c, (off, wd) in enumerate(CH):
                            s6 = spool.tile([P, 6], F32, tag="s6")
                            nc.vector.bn_stats(s6[:wd, :], h[:wd, b, c, :])
                            nc.vector.bn_aggr(mv[:wd, bj, c, :], s6[:wd, :])
                    rstd = spool.tile([P, 2 * NCH], F32, tag="rstd")
                    _rsqrt_newton(nc, spool, rstd,
                                  mv[:].rearrange("p b c s -> p (b c) s")[:, :, 1],
                                  EPS)
                    nmean = spool.tile([P, 2 * NCH], F32, tag="nmean")
                    nc.vector.scalar_tensor_tensor(
                        nmean[:], mv[:].rearrange("p b c s -> p (b c) s")[:, :, 0],
                        -1.0, rstd[:], OP.mult, OP.mult)
                    for bj in range(2):
                        b = 2 * pp + bj
                        for c, (off, wd) in enumerate(CH):
                            xl = apool.tile([P, C], F32R, tag="xln")
                            i = bj * NCH + c
                            nc.scalar.activation(
                                xl[:wd, :], h[:wd, b, c, :], AF.Identity,
                                bias=nmean[:wd, i:i + 1], scale=rstd[:wd, i:i + 1])
                            for f in range(3):
                                pt = ps1.tile([P, 512], F32, tag="ps1")
                                nc.tensor.transpose(
                                    pt[:, :wd].bitcast(F32R),
                                    xl[:wd, f * P:(f + 1) * P],
                                    identr[:wd, :wd])
                                dst = xfm[:, f, b * W + off:b * W + off + wd]
                                if (b * 6 + c * 3 + f) % 2 == 0:
                                    nc.vector.tensor_copy(dst, pt[:, :wd])
                                else:
                                    nc.scalar.copy(dst, pt[:, :wd])

            layernorm_to_xfm(xfm)

            # ---- attention, per image pair ----
            for p in range(PAIRS) if PHASE >= 2 else []:
                b0 = 2 * p
                # Q,K for the pair: [128, 6 mtiles, QK_W]
                qk = qkpool.tile([64, 12, QK_W], F32R, tag="qk")
                nc.vector.memset(qk[:, :, PW:].bitcast(F32), 0.0)
                for m in range(12):
                    pt = ps1.tile([P, 512], F32, tag="ps1")
                    acc = pt[:64, :PW]
                    for kt in range(3):
                        nc.tensor.matmul(acc, wqk_t[:, kt, m * 64:(m + 1) * 64],
                                         xfm[:, kt, b0 * W:b0 * W + PW],
                                         start=(kt == 0), stop=(kt == 2))
                    if m < 6:
                        nc.scalar.activation(qk[:, m, :PW], acc, AF.Identity,
                                             bias=bqk_t[:64, m:m + 1])
                    else:
                        nc.vector.tensor_copy(qk[:, m, :PW], acc)
                # V for both images: token-major [tok, kc, 6*66]
                vts = []
                for bi in (b0, b0 + 1):
                    vt = vpool.tile([P, NCH, 396], F32R, tag="vt")
                    for c, (off, wd) in enumerate(CH):
                        pv = ps1.tile([P, 512], F32, tag="ps1")
                        acc = pv[:wd, :396]
                        for kt in range(3):
                            nc.tensor.matmul(
                                acc, xfm[:, kt, bi * W + off:bi * W + off + wd],
                                wv_t[:, kt, :], start=(kt == 0), stop=(kt == 2))
                        nc.vector.tensor_tensor(vt[:wd, c, :], acc, bv_m[:wd, :],
                                                OP.add)
                    vts.append(vt)

                for bi in ((b0, b0 + 1) if PHASE >= 3 else []):
                    vt = vts[bi - b0]
                    qoff = (bi - b0) * W
                    # scores S^T and exp
                    pts = []
                    for c, (off, wd) in enumerate(CH):
                        sps = ps3.tile([P, 6, 256], F32, tag="ps3")
                        for hh in range(6):
                            nc.tensor.matmul(
                                sps[:wd, hh, :],
                                qk[:, 6 + hh, qoff + off:qoff + off + wd],
                                qk[:, hh, qoff:qoff + 256],
                                start=True, stop=True)
                        pt = ppool.tile([P, 6, 256], F32R, tag="pt")
                        if PHASE >= 4:
                            nc.scalar.activation(
                                pt[:wd, :, :], sps[:wd, :, :], AF.Exp,
                                bias=mb[:wd, bi, c:c + 1], scale=float(SCALE))
                        else:
                            nc.vector.tensor_copy(pt[:wd, :, :].bitcast(F32), sps[:wd, :, :])
                        pts.append(pt)
                    # AV + denominators, per query chunk
                    for qc, (qo, qw) in enumerate(CH) if PHASE >= 5 else []:
                        tps = ps1.tile([P, 512], F32, tag="ps1")
                        tview = tps[:, :396].rearrange("p (h c) -> p h c", c=66)
                        for hh in range(6):
                            for c, (off, wd) in enumerate(CH):
                                nc.tensor.matmul(
                                    tview[:qw, hh, :],
                                    pts[c][:wd, hh, qo:qo + qw],
                                    vt[:wd, c, hh * 66:hh * 66 + 66],
                                    start=(c == 0), stop=(c == 1))
                        r = spool.tile([P, 6], F32, tag="rr")
                        nc.vector.reciprocal(r[:qw, :], tview[:qw, :, 64])
                        ot = opool.tile([P, 6, 64], F32R, tag="ot")
                        for hh in range(6):
                            nc.vector.tensor_scalar_mul(
                                ot[:qw, hh, :], tview[:qw, hh, :64],
                                r[:qw, hh:hh + 1])
                        # prune scores: weighted CLS column of exp tiles
                        if li in PRUNE and qc == 0:
                            wb = prpool.tile([P, 6], F32, tag="wb")
                            nc.gpsimd.partition_broadcast(wb[:], r[0:1, :])
                            sc = prpool.tile([P, NCH], F32, tag="sc")
                            for c, (off, wd) in enumerate(CH):
                                t6 = prpool.tile([P, 6], F32, tag="t6")
                                nc.vector.tensor_tensor(
                                    t6[:wd, :], pts[c][:wd, :, 0], wb[:wd, :],
                                    OP.mult)
                                nc.vector.reduce_sum(
                                    sc[:wd, c:c + 1], t6[:wd, :],
                                    axis=mybir.AxisListType.X)
                                nc.sync.dma_start(
                                    sc_dram.ap()[bi, c * 128:c * 128 + wd],
                                    sc[:wd, c])
                        # O^T -> feature-major via PE, then proj chunk
                        ofm = opool.tile([P, 3, P], F32R, tag="ofm")
                        for f in range(3):
                            pt2 = ps1.tile([P, 512], F32, tag="ps1")
                            nc.tensor.transpose(
                                pt2[:, :qw],
                                ot[:qw, :, :].rearrange("p h d -> p (h d)")
                                [:, f * P:(f + 1) * P].bitcast(F32),
                                ident[:qw, :qw])
                            if f % 2 == 0:
                                nc.vector.tensor_copy(ofm[:, f, :qw], pt2[:, :qw])
                            else:
                                nc.scalar.copy(ofm[:, f, :qw], pt2[:, :qw])
                        pj = ps1.tile([P, 512], F32, tag="ps1")
                        acc = pj[:qw, :C]
                        for kt in range(3):
                            nc.tensor.matmul(acc, ofm[:, kt, :qw], wp_t[:, kt, :],
                                             start=(kt == 0), stop=False)
                        nc.tensor.matmul(acc, ones1[:, :qw], bp_t[:],
                                         start=False, stop=True)
                        nc.vector.tensor_tensor(h[:qw, bi, qc, :],
                                                h[:qw, bi, qc, :], acc, OP.add)

            # ---- prune mask update ----
            if li in PRUNE:
                drop = PRUNE[li]
                scm = prpool.tile([IMGS, NCH * 128], F32, tag="scm")
                nc.sync.dma_start(scm[:], sc_dram.ap())
                # t = -1e9*(sc==0) - sc  over tokens 1..196
                tneg = prpool.tile([IMGS, NCH * 128], F32, tag="tneg")
                u = prpool.tile([IMGS, NCH * 128], F32, tag="uu")
                nc.vector.tensor_scalar(u[:, 1:NTOK], scm[:, 1:NTOK], 0.0, None,
                                        OP.is_equal)
                nc.vector.scalar_tensor_tensor(
                    tneg[:, 1:NTOK], u[:, 1:NTOK], -1e9, scm[:, 1:NTOK],
                    OP.mult, OP.subtract)
                m8 = prpool.tile([IMGS, 8], F32, tag="m8")
                left = drop
                while left > 0:
                    k = min(8, left)
                    nc.vector.max(m8[:], tneg[:, 1:NTOK])
                    if k < 8:
                        nc.vector.memset(m8[:, k:], -2e30)
                    nc.vector.match_replace(tneg[:, 1:NTOK], m8[:],
                                            tneg[:, 1:NTOK], NEG)
                    left -= k
                dm = prpool.tile([IMGS, NCH * 128], F32, tag="dm")
                nc.vector.memset(dm[:], 0.0)
                nc.vector.tensor_scalar(dm[:, 1:NTOK], tneg[:, 1:NTOK], -1e29,
                                        None, OP.is_le)
                nc.sync.dma_start(dm_dram.ap(), dm[:])
                dmc = prpool.tile([P, IMGS, NCH], F32, tag="dmc")
                for b in range(IMGS):
                    nc.sync.dma_start(
                        dmc[:, b, :],
                        bass.AP(dm_dram, b * NCH * 128, [[1, 128], [128, NCH]]))
                nc.vector.scalar_tensor_tensor(mb[:], dmc[:], NEG, mb[:],
                                               OP.mult, OP.add)

            # ---- LN2 + transpose (reuse xfm) ----
            if PHASE >= 6:
                xfm2 = xpool.tile([P, 3, TPAD], F32R, tag="xfm")
                layernorm_to_xfm(xfm2)

            # ---- MLP per pair ----
            for p in range(PAIRS) if PHASE >= 6 else []:
                b0 = 2 * p
                g = gpool.tile([P, 12, PW], F32R, tag="g")
                for m in range(12):
                    f1 = ps1.tile([P, 512], F32, tag="ps1")
                    acc = f1[:, :PW]
                    for kt in range(3):
                        nc.tensor.matmul(
                            acc, w1_t[:, kt, m * P:(m + 1) * P],
                            xfm2[:, kt, b0 * W:b0 * W + PW],
                            start=(kt == 0), stop=(kt == 2))
                    nc.scalar.activation(g[:, m, :], acc, AF.Gelu,
                                         bias=b1f_t[:, m:m + 1])
                for bi in (b0, b0 + 1):
                    for c, (off, wd) in enumerate(CH):
                        span = (bi - b0) * W + off
                        f2 = ps1.tile([P, 512], F32, tag="ps1")
                        acc = f2[:wd, :C]
                        for kt in range(12):
                            nc.tensor.matmul(acc, g[:, kt, span:span + wd],
                                             w2_t[:, kt, :],
                                             start=(kt == 0), stop=False)
                        nc.tensor.matmul(acc, ones1[:, :wd], b2_t[:],
                                         start=False, stop=True)
                        nc.vector.tensor_tensor(h[:wd, bi, c, :],
                                                h[:wd, bi, c, :], acc, OP.add)

        # ---------------- final LN + head ----------------
        if debug_h:
            nc.sync.dma_start(hdbg.ap(), h[:].rearrange("p b c f -> p b c f"))
        for b in range(IMGS):
            nc.sync.dma_start(cls_dram.ap()[b, :], h[0:1, b, 0, :])
        clst = prpool.tile([IMGS, C], F32, tag="clst")
        nc.sync.dma_start(clst[:], cls_dram.ap())
        s6 = prpool.tile([IMGS, 6], F32, tag="s6f")
        mv = prpool.tile([IMGS, 2], F32, tag="mvf")
        nc.vector.bn_stats(s6[:], clst[:])
        nc.vector.bn_aggr(mv[:], s6[:])
        rstd = prpool.tile([IMGS, 1], F32, tag="rstdf")
        _rsqrt_newton(nc, spool, rstd, mv[:, 1:2], EPS)
        nmean = prpool.tile([IMGS, 1], F32, tag="nmeanf")
        nc.vector.scalar_tensor_tensor(nmean[:], mv[:, 0:1], -1.0, rstd[:],
                                       OP.mult, OP.mult)
        clsn = prpool.tile([IMGS, C], F32R, tag="clsn")
        nc.scalar.activation(clsn[:], clst[:], AF.Identity, bias=nmean[:],
                             scale=rstd[:])
        clsf = prpool.tile([P, 3, IMGS], F32R, tag="clsf")
        for f in range(3):
            pt = ps1.tile([P, 512], F32, tag="ps1")
            nc.tensor.transpose(pt[:, :IMGS],
                                clsn[:, f * P:(f + 1) * P].bitcast(F32),
                                ident[:IMGS, :IMGS])
            nc.vector.tensor_copy(clsf[:, f, :], pt[:, :IMGS])
        wh_t = prpool.tile([P, 3, NCLS], F32R, tag="wht")
        nc.sync.dma_start(wh_t[:], wh.ap().rearrange("k p m -> p k m"))
        bh_t = prpool.tile([1, NCLS], F32R, tag="bht")
        nc.sync.dma_start(bh_t[:], bh.ap())
        po = ps1.tile([P, 512], F32, tag="ps1")
        acc = po[:IMGS, :NCLS]
        for kt in range(3):
            nc.tensor.matmul(acc, clsf[:, kt, :], wh_t[:, kt, :],
                             start=(kt == 0), stop=False)
        nc.tensor.matmul(acc, ones1[:, :IMGS], bh_t[:], start=False, stop=True)
        ot = prpool.tile([IMGS, NCLS], F32, tag="outf")
        nc.vector.tensor_copy(ot[:], acc)
        nc.sync.dma_start(out.ap(), ot[:])

    nc.finalize()
    return nc


# ======================= host side =======================

def _prep(inputs):
    """Host-side: patchify x, fold LN affines, lay out weights."""
    f32 = np.float32
    d = {}
    x = np.asarray(inputs["x"], f32)
    Bn = x.shape[0]
    # patches feature-major, with token shift (col 0 = CLS placeholder)
    p = x.reshape(Bn, 3, 14, 16, 14, 16).transpose(0, 2, 4, 1, 3, 5)
    p = p.reshape(Bn, NPATCH, 768)
    xp = np.zeros((Bn, 768, W), f32)
    xp[:, :, 1:NTOK] = p.transpose(0, 2, 1)
    d["xp_all"] = xp.reshape(Bn, 6, 128, W)

    pw_ = np.asarray(inputs["patch_w"], f32)
    d["pw"] = pw_.reshape(6, 128, C)

    h0b = np.zeros((NCH, 128, C), f32)
    pos = np.asarray(inputs["pos_embed"], f32)[0]
    pb = np.asarray(inputs["patch_b"], f32)
    cls0 = np.asarray(inputs["cls_token"], f32).reshape(C) + pos[0]
    bias_tok = np.zeros((W, C), f32)
    bias_tok[0] = cls0
    bias_tok[1:NTOK] = pb[None, :] + pos[1:]
    for c, (off, wd) in enumerate(CH):
        h0b[c, :wd] = bias_tok[off:off + wd]
    d["h0b"] = h0b

    mb_ = np.zeros((NCH, 128), f32)
    for c, (off, wd) in enumerate(CH):
        for pp in range(128):
            t = off + pp
            if pp >= wd or t >= NTOK:
                mb_[c, pp] = NEG
    d["mb0"] = mb_

    qkv_w = np.asarray(inputs["qkv_w"], f32)
    qkv_b = np.asarray(inputs["qkv_b"], f32)
    g1 = np.asarray(inputs["ln1_g"], f32)
    b1_ = np.asarray(inputs["ln1_b"], f32)
    g2 = np.asarray(inputs["ln2_g"], f32)
    b2_ = np.asarray(inputs["ln2_b"], f32)

    wqk_l = np.zeros((LAYERS, 3, 128, 768), f32)
    bqk_l = np.zeros((LAYERS, 12, 64), f32)
    wv_l = np.zeros((LAYERS, 3, 128, 396), f32)
    bv_l = np.zeros((LAYERS, 1, 396), f32)
    for li in range(LAYERS):
        wq = qkv_w[li] * g1[li][:, None]          # [C, 3C] folded
        bq = qkv_b[li] + b1_[li] @ qkv_w[li]
        wqk_l[li] = wq[:, :768].reshape(3, 128, 768)
        bqk_l[li] = bq[:768].reshape(12, 64)
        wvl = np.zeros((C, 396), f32)
        bvl = np.zeros((396,), f32)
        for hh in range(HEADS):
            wvl[:, hh * 66:hh * 66 + 64] = wq[:, 768 + hh * 64:768 + (hh + 1) * 64]
            bvl[hh * 66:hh * 66 + 64] = bq[768 + hh * 64:768 + (hh + 1) * 64]
            bvl[hh * 66 + 64] = 1.0
        wv_l[li] = wvl.reshape(3, 128, 396)
        bv_l[li, 0] = bvl
    d["wqk"], d["bqk"], d["wv"], d["bv"] = wqk_l, bqk_l, wv_l, bv_l

    d["wp"] = np.asarray(inputs["proj_w"], f32).reshape(LAYERS, 3, 128, C)
    d["bp"] = np.asarray(inputs["proj_b"], f32).reshape(LAYERS, 1, C)
    w1_ = np.asarray(inputs["fc1_w"], f32) * g2[:, :, None]
    d["w1"] = w1_.reshape(LAYERS, 3, 128, MLP)
    d["b1"] = (np.asarray(inputs["fc1_b"], f32)
               + np.einsum('lc,lcm->lm', b2_, np.asarray(inputs["fc1_w"], f32))
               ).reshape(LAYERS, 1, MLP)
    d["w2"] = np.asarray(inputs["fc2_w"], f32).reshape(LAYERS, 12, 128, C)
    d["b2"] = np.asarray(inputs["fc2_b"], f32).reshape(LAYERS, 1, C)

    ng = np.asarray(inputs["norm_g"], f32)
    nb = np.asarray(inputs["norm_b"], f32)
    hw = np.asarray(inputs["head_w"], f32)
    d["wh"] = (hw * ng[:, None]).reshape(3, 128, NCLS)
    d["bh"] = (np.asarray(inputs["head_b"], f32) + nb @ hw).reshape(1, NCLS)
    return d


_NC_CACHE = {}


def kernel(**inputs):
    key = (N_LAYERS_BUILD, DEBUG_H, PHASE)
    if key not in _NC_CACHE:
        _NC_CACHE[key] = build_kernel()
    nc = _NC_CACHE[key]
    d = _prep(inputs)
    shared = {k: np.ascontiguousarray(v) for k, v in d.items() if k != "xp_all"}
    in_maps = []
    for core in range(8):
        m = dict(shared)
        m["xp"] = np.ascontiguousarray(
            d["xp_all"][core * IMGS:(core + 1) * IMGS].transpose(1, 2, 0, 3))
        in_maps.append(m)
    res = run_bass_kernel_spmd(nc, in_maps, core_ids=list(range(8)))
    outs = [r["out"] for r in res.results]
    return np.concatenate(outs, axis=0)


if __name__ == "__main__":
    rng = np.random.default_rng(0)
    print("building kernel ...")
    nc = build_kernel(1)
    print("built OK")

